# revision 22
# baseline (speedup 1.0000x reference)
"""CrossAttention Trainium2 kernel (Bass/Tile), 8-core SPMD.

Problem: q = query@Wq+bq; k = key@Wk+bk; v = value@Wv+bv;
         out = softmax(q k^T) v           (no 1/sqrt(d) scaling)
Shapes:  query [4, 2048, 1024], key/value [4, 2048, 768],
         W* [(1024|768), 1024], b* [1024], out [4, 2048, 1024] f32.

Sharding: data-parallel over (batch, query-half) -> 8 shards of 1024 query
rows per core. No collectives.

Algebraic restructuring (the big win): softmax is invariant to per-row
constants, so
  scores  = (x Wq + bq)(y Wk + bk)^T
         == x (Wq Wk^T) y^T + t[j],   t = key @ (Wk @ bq)   (host-computed)
which deletes both the Q and K projections (the H=1024 contraction
collapses into the host-precomputed Wqk, and the scores contraction
shrinks H=1024 -> D2=768). The V side is re-associated as
  out = (probs @ value) @ Wv + bv
so the Wv GEMM runs on [M=1024, 768] instead of [LK=2048, 768].
Per-core PE work drops from ~218us (baseline) to ~123us.

Precision: scores path f32r end-to-end (logits are sigma~32; bf16 there
costs ~0.1 logit of noise and softmax ties amplify it). V path bf16
(fp8 measured at rel-err 4.6e-2 - the values' own quantization noise -
so fp8 is banned everywhere).

Stage D is software-pipelined: AVy of m-tile i is emitted after scores of
m-tile i+2, AVw after scores of m-tile i+3, giving the softmax->transpose
->AVy->transpose chains multiple score-windows of slack.
"""

import os
import sys
from contextlib import ExitStack

for _p in ("/opt/trn_rl_repo", "/root/.axon_site/_ro/trn_rl_repo"):
    if os.path.isdir(_p) and _p not in sys.path:
        sys.path.append(_p)

import numpy as np

import concourse.bass as bass
import concourse.mybir as mybir
import concourse.tile as tile
from concourse import bacc
from concourse.bass import ts
from concourse.bass_utils import run_bass_kernel_spmd

P = 128
B, LQ, LK = 4, 2048, 2048
D1, D2, H = 1024, 768, 1024
N_CORES = 8
M = (B * LQ) // N_CORES  # 1024 query rows per core

D1T, D2T, HT = D1 // P, D2 // P, H // P
MT, JT, JC, MC = M // P, LK // P, LK // 512, M // 512
ET = D2 // P  # 6 tiles of the D2 contraction/intermediate axis

F32 = mybir.dt.float32
F32R = mybir.dt.float32r
BF16 = mybir.dt.bfloat16
AX = mybir.AxisListType.X
AF = mybir.ActivationFunctionType
ALU = mybir.AluOpType

_CACHE = {}
LAST_RESULTS = None  # BassKernelResults of the most recent run (for test harness)


def _build_bass():
    nc = bacc.Bacc("TRN2", target_bir_lowering=False, debug=False,
                   num_devices=N_CORES)

    # Host-pre-tiled operands: every DMA chunk below is contiguous.
    xq = nc.dram_tensor("xq", [P, MC, D1T, 512], F32R, kind="ExternalInput")
    wqkd = nc.dram_tensor("wqk", [P, ET, D1T, P], F32R, kind="ExternalInput")
    kyd = nc.dram_tensor("ky", [P, JC, ET, 512], F32R, kind="ExternalInput")
    yvd = nc.dram_tensor("yv", [P, JT, D2], BF16, kind="ExternalInput")
    wvd = nc.dram_tensor("wv", [P, ET, H], BF16, kind="ExternalInput")
    tqd = nc.dram_tensor("tq", [LK], BF16, kind="ExternalInput")
    bvd = nc.dram_tensor("bv", [H], F32, kind="ExternalInput")
    out = nc.dram_tensor("out", [M, H], F32, kind="ExternalOutput")

    with tile.TileContext(nc) as tc, ExitStack() as top:
        # Shared PSUM pools: "acc" for scores/Z/AVw, py1/py2 for AVy.
        pps = top.enter_context(tc.tile_pool(name="pps", bufs=4, space="PSUM"))
        ppy = top.enter_context(tc.tile_pool(name="ppy", bufs=2, space="PSUM"))

        # Residents: zT [768, M] f32r, yT [768, LK] f32r, yv [LK, 768] bf16,
        # wv [768, H] bf16, t/bv broadcasts.
        respool = top.enter_context(tc.tile_pool(name="res", bufs=1))
        zT = respool.tile([P, ET, M], F32R)
        yTs = respool.tile([P, ET, LK], F32R)
        yv = respool.tile([P, JT, D2], BF16)
        wvs = respool.tile([P, ET, H], BF16)
        tsb = respool.tile([P, JC, 512], BF16)
        bv_full = respool.tile([P, H], F32)

        # ---- Stage Z: zT[e, m] = Wqk^T @ X^T  (f32r) ----
        # Need-ordered queue: xq halves + per-et Wqk chunks feed the PE from
        # ~17us; yT/yv/wv stream in behind while Z and early scores run.
        with tc.tile_pool(name="sa1", bufs=1, side="left") as sa1, \
                tc.tile_pool(name="saw", bufs=6, side="left") as saw:
            xTs = sa1.tile([P, MC, D1T, 512], F32R)
            nc.sync.dma_start(xTs[:, 0], xq[:, 0])
            wqkcs = []
            for et in range(ET):
                wqkc = saw.tile([P, D1T, P], F32R, tag="wqkc")
                nc.sync.dma_start(wqkc[:], wqkd[:, et])
                wqkcs.append(wqkc)
            nc.sync.dma_start(xTs[:, 1], xq[:, 1])
            # Stage-D operand stream queued behind Z's operands, in need order.
            for jc in range(3):
                nc.sync.dma_start(yTs[:, :, ts(jc, 512)], kyd[:, jc])
            nc.sync.dma_start(tsb[:].rearrange("p a b -> p (a b)"),
                              tqd[None, :].to_broadcast([P, LK]))
            nc.sync.dma_start(yTs[:, :, ts(3, 512)], kyd[:, 3])
            for c4 in range(4):
                nc.sync.dma_start(yv[:, 4 * c4:4 * c4 + 4, :],
                                  yvd[:, 4 * c4:4 * c4 + 4, :])
            nc.sync.dma_start(wvs[:], wvd[:])
            nc.sync.dma_start(bv_full[:], bvd[None, :].to_broadcast([P, H]))
            for mc in range(MC):
                for et in range(ET):
                    psz = pps.tile([P, 512], F32, tag="acc")
                    for dt in range(D1T):
                        nc.tensor.matmul(psz[:], wqkcs[et][:, dt, :],
                                         xTs[:, mc, dt, :],
                                         start=(dt == 0), stop=(dt == D1T - 1))
                    nc.scalar.activation(zT[:, et, ts(mc, 512)], psz[:],
                                         AF.Copy, scale=1.0)

        # ---- Stage D pools ----
        esD = top.enter_context(ExitStack())
        sd2 = esD.enter_context(tc.tile_pool(name="sd2", bufs=3, side="right"))
        sd3 = esD.enter_context(tc.tile_pool(name="sd3", bufs=3, side="right"))
        sdz = esD.enter_context(tc.tile_pool(name="sdz", bufs=3, side="right"))
        stat = esD.enter_context(tc.tile_pool(name="stat", bufs=4,
                                              side="right"))

        # ---- Stage D: scores -> softmax -> (probs @ value) @ Wv ----
        def scores_softmax(mt):
            ssb = sd2.tile([P, JC, 512], F32, tag="ssb")
            mx4 = stat.tile([P, JC], F32, tag="mx4")
            for jp in range(JC // 2):
                pss = [pps.tile([P, 512], F32, tag="acc", name=f"pss{h_}")
                       for h_ in range(2)]
                for et in range(ET):
                    for half in range(2):
                        nc.tensor.matmul(pss[half][:], zT[:, et, ts(mt, P)],
                                         yTs[:, et, ts(2 * jp + half, 512)],
                                         start=(et == 0), stop=(et == ET - 1))
                for half in range(2):
                    jc = 2 * jp + half
                    # copy + fold the per-key bias t[j] in one DVE pass
                    nc.vector.tensor_tensor(ssb[:, jc, :], pss[half][:],
                                            tsb[:, jc, :], ALU.add)
                    nc.vector.reduce_max(mx4[:, jc:jc + 1], ssb[:, jc, :],
                                         axis=AX)
            negmax = stat.tile([P, 1], F32, tag="negmax")
            nc.vector.reduce_max(negmax[:], mx4[:], axis=AX, negate=True)
            wsb = sd2.tile([P, JC, 512], BF16, tag="wsb")
            sm4 = stat.tile([P, JC], F32, tag="sm4")
            for jc in range(JC):
                nc.scalar.activation(wsb[:, jc, :], ssb[:, jc, :], AF.Exp,
                                     bias=negmax[:, 0:1], scale=1.0,
                                     accum_out=sm4[:, jc:jc + 1])
            ssum = stat.tile([P, 1], F32, tag="ssum")
            nc.vector.reduce_sum(ssum[:], sm4[:], axis=AX)
            rinv = stat.tile([P, 1], F32, tag="rinv")
            nc.vector.reciprocal(rinv[:], ssum[:])
            wT = sd3.tile([P, JT, P], BF16, tag="wT")
            nc.sync.dma_start_transpose(
                wT[:], wsb[:].rearrange("p a b -> p (a b)"))
            return wT, rinv

        def avy(mt, wT):
            # zy[m, e] = sum_j probs^T[j, m] * value[j, e]   (bf16)
            # 384/384 split keeps every matmul >= LDWEIGHTS time.
            py1 = ppy.tile([P, 384], F32, tag="py1")
            py2 = ppy.tile([P, 384], F32, tag="py2")
            for jt in range(JT):
                nc.tensor.matmul(py1[:], wT[:, jt, :], yv[:, jt, 0:384],
                                 start=(jt == 0), stop=(jt == JT - 1))
                nc.tensor.matmul(py2[:], wT[:, jt, :], yv[:, jt, 384:D2],
                                 start=(jt == 0), stop=(jt == JT - 1))
            zy = sdz.tile([P, D2], BF16, tag="zy")
            nc.vector.tensor_copy(zy[:, 0:384], py1[:])
            nc.vector.tensor_copy(zy[:, 384:D2], py2[:])
            zyT = sdz.tile([P, ET, P], BF16, tag="zyT")
            nc.sync.dma_start_transpose(zyT[:], zy[:])
            return zyT

        def avw(mt, zyT, rinv):
            # Finer epilogue chunks on the last tile shorten the end-of-kernel
            # scale->bias->DMA chain.
            ec = 256 if mt == MT - 1 else 512
            osb = sd2.tile([P, H], F32, tag="osb")
            for hc in range(H // 512):
                psa = pps.tile([P, 512], F32, tag="acc")
                for et in range(ET):
                    nc.tensor.matmul(psa[:], zyT[:, et, :],
                                     wvs[:, et, ts(hc, 512)],
                                     start=(et == 0), stop=(et == ET - 1))
                for k in range(512 // ec):
                    sl = slice(hc * 512 + k * ec, hc * 512 + (k + 1) * ec)
                    nc.scalar.activation(osb[:, sl],
                                         psa[:, k * ec:(k + 1) * ec],
                                         AF.Copy, scale=rinv[:, 0:1])
                    nc.vector.tensor_tensor(osb[:, sl], osb[:, sl],
                                            bv_full[:, sl], ALU.add)
                    nc.sync.dma_start(out[ts(mt, P), sl], osb[:, sl])

        penda = []  # (mt, wT, rinv) awaiting AVy
        pendw = []  # (mt, zyT, rinv) awaiting AVw
        for mt in range(MT):
            penda.append((mt,) + scores_softmax(mt))
            if len(penda) > 2:
                amt, wT, rinv = penda.pop(0)
                pendw.append((amt, avy(amt, wT), rinv))
            if len(pendw) > 1:
                wmt, zyT, rinv = pendw.pop(0)
                avw(wmt, zyT, rinv)
        while penda or pendw:
            if penda:
                amt, wT, rinv = penda.pop(0)
                pendw.append((amt, avy(amt, wT), rinv))
            wmt, zyT, rinv = pendw.pop(0)
            avw(wmt, zyT, rinv)

    nc.compile()
    return nc


def _get_nc():
    if "nc" not in _CACHE:
        _CACHE["nc"] = _build_bass()
    return _CACHE["nc"]


def kernel(query, key, value, Wq, bq, Wk, bk, Wv, bv):
    global LAST_RESULTS
    nc = _get_nc()
    import ml_dtypes

    def f(a):
        return np.ascontiguousarray(np.asarray(a, dtype=np.float32))

    query, key, value = f(query), f(key), f(value)
    Wq, bq, Wk, bk, Wv, bv = f(Wq), f(bq), f(Wk), f(bk), f(Wv), f(bv)

    # Host-side algebra (f64): Wqk = Wq Wk^T ; t = key @ (Wk bq).
    Wqk = (Wq.astype(np.float64) @ Wk.astype(np.float64).T).astype(np.float32)
    wkbq = Wk.astype(np.float64) @ bq.astype(np.float64)  # [D2]
    tq = [(key[b].astype(np.float64) @ wkbq).astype(ml_dtypes.bfloat16)
          for b in range(B)]

    def tile_lhs(w, kt, nb):  # [K, N] -> [P, nb, kt, N/nb] chunk-contiguous
        n = w.shape[1]
        return np.ascontiguousarray(
            w.reshape(kt, P, nb, n // nb).transpose(1, 2, 0, 3))

    wqk_t = tile_lhs(Wqk, D1T, ET)                     # [P, ET, D1T, 128]
    wv_t = np.ascontiguousarray(
        Wv.astype(ml_dtypes.bfloat16).reshape(ET, P, H).transpose(1, 0, 2))

    half = LQ // 2
    ky_t = [tile_lhs(key[b].T.copy(), ET, JC) for b in range(B)]
    yv_t = [np.ascontiguousarray(
        value[b].astype(ml_dtypes.bfloat16).reshape(JT, P, D2)
        .transpose(1, 0, 2)) for b in range(B)]

    in_maps = []
    for c in range(N_CORES):
        b, hh = divmod(c, 2)
        xqT = query[b, hh * half:(hh + 1) * half, :].T  # [D1, M]
        xq_t = np.ascontiguousarray(
            xqT.reshape(D1T, P, MC, 512).transpose(1, 2, 0, 3))
        in_maps.append({
            "xq": xq_t, "wqk": wqk_t, "ky": ky_t[b], "yv": yv_t[b],
            "wv": wv_t, "tq": tq[b], "bv": bv,
        })

    res = run_bass_kernel_spmd(nc, in_maps, core_ids=list(range(N_CORES)))
    LAST_RESULTS = res

    out = np.empty((B, LQ, H), dtype=np.float32)
    for c in range(N_CORES):
        b, hh = divmod(c, 2)
        out[b, hh * half:(hh + 1) * half, :] = res.results[c]["out"]
    return out
